# revision 9
# baseline (speedup 1.0000x reference)
"""Trainium2 Bass kernel for nn_AdaptiveMixedCoding (8 NeuronCores).

Sharding: data-parallel over B_img (8 images per core); caps/cap_lens/alpha
replicated. Caption Grams computed split across cores (8 caps each, batched
2-at-a-time) and AllGathered.

Per-core algorithm (Bi=8 imgs, R=36 regions, Bc=64 caps, W=50 words, D=1024):
  S[row, c, w] = dot(imgs[row], caps[c, w]) + adds   (fp16 matmul, K=1 ones
                 row accumulates 0 valid / -30000 masked into same PSUM group)
  t            = S_sb * bc_scale'  (fp16; bc_scale' = inv_nc valid / 0.01
                 masked -> masked t ~= -300, fp16-safe)
  exp          = Exp(t*invni10 + (-rowmax_all*invni10))   per-ROW max folded
                 into the scalar-engine bias (per-caption max cancels in the
                 softmax), so no per-caption subtract pass is needed
  hard         = (t == rowmax_c)   per-caption fp16 max, exact compare
  mixed'       = hard + exp * a/((1-a) den)            (= mixed/(1-a))
  num'         = sum_w mixed' * S_sb
  qf'          = mixed'^T G mixed'  (pair transposes -> M_T, u = M_T^T Gp,
                 4 pairs packed per PSUM bank, fused products, one reduce)
  out          = num'/(sqrt(qf') + eps/(1-a)), invalid img rows -> -1

Row tiles 128/128/32 (full partition use). DMA order: gcaps, imgsT, caps by
column-chunks, imgs_nat last -> grams + S matmuls start early; Gram
AllGather triggers ~10us in and overlaps the S phase.

End-to-end l2 rel err vs the f32 reference ~8e-3 (fp16 t argmax ties).
"""
import sys
import contextlib

sys.path.insert(0, '/opt/trn_rl_repo')

import numpy as np
import ml_dtypes

from concourse import bacc, tile, mybir

F32 = mybir.dt.float32
F16 = mybir.dt.float16
AF = mybir.ActivationFunctionType
OP = mybir.AluOpType
AX = mybir.AxisListType

N_CORES = 8
B, R, W, D = 64, 36, 50, 1024
BC = B
BI = B // N_CORES
ROWS = BI * R               # 288
CW = BC * W                 # 3200
KC = D // 128               # 8 contraction chunks
NP = BC // 2                # 32 caption pairs
PPC = NP // N_CORES         # 4 pairs (8 captions) per core
CPC = BC // N_CORES         # 8 captions per core
GW = PPC * 128              # padded gram columns per core (50|pad|50 per pair)
ROW_TILES = [(0, 128), (128, 128), (256, 32)]
N_CHUNKS = [(i * 512, min(512, CW - i * 512)) for i in range((CW + 511) // 512)]
EPS = 1e-8
NEGS = -30000.0             # masked S offset; fp16-safe, *0.01 -> t ~= -300
KMASK = 0.01
TINY = 1e-30

_CACHE = {}


def _build(a: float):
    am = max(a, 1e-6)
    oma = max(1.0 - a, 1e-6)

    nc = bacc.Bacc("TRN2", target_bir_lowering=False, debug=False,
                   num_devices=N_CORES)

    capsT = nc.declare_dram_parameter("capsT", [D, CW], F16, isOutput=False)
    gcaps = nc.declare_dram_parameter("gcaps", [D, GW], F16, isOutput=False)
    imgsT = nc.declare_dram_parameter("imgsT", [D, ROWS], F16, isOutput=False)
    imgs_nat = nc.declare_dram_parameter("imgs_nat", [ROWS, D], F32,
                                         isOutput=False)
    bc_scale_in = nc.declare_dram_parameter("bc_scale_in", [128, CW], F16,
                                            isOutput=False)  # invnc / KMASK
    adds_row = nc.declare_dram_parameter("adds_row", [1, CW], F16,
                                         isOutput=False)    # 0 / NEGS
    iv_col = nc.declare_dram_parameter("iv_col", [ROWS, 1], F32,
                                       isOutput=False)
    ivm1_col = nc.declare_dram_parameter("ivm1_col", [ROWS, 1], F32,
                                         isOutput=False)
    out_ext = nc.declare_dram_parameter("out", [BI, BC, R], F32, isOutput=True)

    gb_in = nc.dram_tensor("gb_in", [PPC, 100, 128], F16)
    gb_out = nc.dram_tensor("gb_out", [NP, 100, 128], F16,
                            addr_space="Shared")

    with tile.TileContext(nc) as tc, contextlib.ExitStack() as ctx:
        const = ctx.enter_context(tc.tile_pool(name="const", bufs=1))
        big = ctx.enter_context(tc.tile_pool(name="big", bufs=1))
        work = ctx.enter_context(tc.tile_pool(name="work", bufs=2))
        work3 = ctx.enter_context(tc.tile_pool(name="work3", bufs=3))
        scr = ctx.enter_context(tc.tile_pool(name="scr", bufs=1))
        small = ctx.enter_context(tc.tile_pool(name="small", bufs=2))
        psS = ctx.enter_context(tc.tile_pool(name="psS", bufs=2, space="PSUM"))
        psQ = ctx.enter_context(tc.tile_pool(name="psQ", bufs=2, space="PSUM"))
        psM = ctx.enter_context(tc.tile_pool(name="psM", bufs=2, space="PSUM"))
        psT = ctx.enter_context(tc.tile_pool(name="psT", bufs=2, space="PSUM"))

        # ---- constants --------------------------------------------------
        ident_16 = const.tile([128, 128], F16)
        from concourse.masks import make_identity
        make_identity(nc, ident_16[:])
        ident_f32 = const.tile([128, 128], F32)
        make_identity(nc, ident_f32[:])
        ones_16 = const.tile([1, 128], F16)
        nc.gpsimd.memset(ones_16[:], 1.0)

        # ---- input loads (priority order) -------------------------------
        gcaps_sb = big.tile([128, KC, GW], F16)
        gcaps_r = gcaps.rearrange("(k p) m -> p k m", p=128)
        for j in range(PPC):
            nc.sync.dma_start(out=gcaps_sb[:, :, 128 * j:128 * j + 128],
                              in_=gcaps_r[:, :, 128 * j:128 * j + 128])
        imgsT_sb = big.tile([128, KC, ROWS], F16)
        nc.sync.dma_start(out=imgsT_sb[:],
                          in_=imgsT.rearrange("(k p) m -> p k m", p=128))
        caps_sb = big.tile([128, KC, CW], F16)
        capsT_r = capsT.rearrange("(k p) m -> p k m", p=128)
        for (n0, nw) in N_CHUNKS:
            nc.sync.dma_start(out=caps_sb[:, :, n0:n0 + nw],
                              in_=capsT_r[:, :, n0:n0 + nw])

        addsrow_sb = const.tile([1, CW], F16)
        nc.gpsimd.dma_start(out=addsrow_sb[:], in_=adds_row[:])

        # ---- Grams for this core's 8 captions (2 caps per matmul) -------
        # Gloc: even cap at [0:50, j, 0:50], odd cap at [64:114, j, 50:100]
        Gloc = big.tile([128, PPC, 128], F16)
        nc.vector.memset(Gloc[:], 0.0)
        for j in range(PPC):
            c0 = j * 128
            gps = psM.tile([128, 128], F32, tag="ps")
            for kc in range(KC):
                nc.tensor.matmul(gps[:, :],
                                 gcaps_sb[:, kc, c0:c0 + 128],
                                 gcaps_sb[:, kc, c0:c0 + 128],
                                 start=(kc == 0), stop=(kc == KC - 1))
            nc.scalar.activation(Gloc[0:50, j, 0:50], gps[0:50, 0:50],
                                 AF.Copy)
            nc.scalar.activation(Gloc[64:114, j, 50:100], gps[64:114, 64:114],
                                 AF.Copy)

        # full Gram gather (overlaps the S matmuls; needed only by qf)
        zb = const.tile([128, PPC * 100], F16)
        nc.vector.memset(zb[:], 0.0)
        nc.gpsimd.dma_start(
            out=gb_in.rearrange("j r b -> (j r b)")[None, :],
            in_=zb[:])
        nc.gpsimd.dma_start(
            out=gb_in[:, 0:50, 0:50].rearrange("j r b -> r j b"),
            in_=Gloc[0:50, :, 0:50])
        nc.gpsimd.dma_start(
            out=gb_in[:, 50:100, 50:100].rearrange("j r b -> r j b"),
            in_=Gloc[64:114, :, 50:100])
        nc.gpsimd.collective_compute(
            "AllGather", OP.bypass,
            replica_groups=[list(range(N_CORES))],
            ins=[gb_in[:].opt()],
            outs=[gb_out[:].opt()],
        )
        # Gp[:, p, :]: G_{2p} at [0:50, 0:50], G_{2p+1} at [50:100, 50:100]
        Gp = big.tile([128, NP, 128], F16)
        nc.vector.memset(Gp[:], 0.0)
        for k in range(N_CORES):
            nc.sync.dma_start(
                out=Gp[0:100, k * PPC:(k + 1) * PPC, :],
                in_=gb_out[k * PPC:(k + 1) * PPC, :, :].rearrange(
                    "j r b -> r j b"))

        # transposed mixed, pair-block layout (built per row tile)
        M_T = big.tile([128, NP, ROWS], F16)
        nc.vector.memset(M_T[:, NP - 1, :], 0.0)

        # bc_scale comes host-prebroadcast [128, CW]
        bc_scale = big.tile([128, CW], F16)
        nc.sync.dma_start(out=bc_scale[:], in_=bc_scale_in[:])

        # persistent output accumulator [BC, ROWS]
        out_sb = big.tile([BC, ROWS], F32)

        # ---- pipelined per-row-tile phases ------------------------------
        def s_phase(r0, rt):
            """S matmul + bias; evac fp16 S_sb; t = S_sb * bc_scale."""
            mm = 128 if (r0 + 128 <= ROWS) else rt
            img_nat_t = work.tile([128, D], F32, tag="imgnat")
            nc.sync.dma_start(out=img_nat_t[:rt, :],
                              in_=imgs_nat[r0:r0 + rt, :])
            sq_scr = scr.tile([128, D], F32, tag="sqscr")
            nsq_img = small.tile([128, 1], F32, tag="nsqimg")
            nc.scalar.activation(sq_scr[:rt, :], img_nat_t[:rt, :], AF.Square,
                                 accum_out=nsq_img[:rt, :])
            invni10 = small.tile([128, 1], F32, tag="invni10")
            nc.scalar.activation(invni10[:rt, :], nsq_img[:rt, :], AF.Sqrt,
                                 scale=0.01)
            nc.vector.reciprocal(invni10[:rt, :], invni10[:rt, :])
            iv_t = small.tile([128, 1], F32, tag="ivt")
            nc.gpsimd.dma_start(out=iv_t[:rt, :], in_=iv_col[r0:r0 + rt, :])
            ivm1_t = small.tile([128, 1], F32, tag="ivm1t")
            nc.gpsimd.dma_start(out=ivm1_t[:rt, :],
                                in_=ivm1_col[r0:r0 + rt, :])

            t = work3.tile([128, CW], F16, tag="t")
            S_sb = work3.tile([128, CW], F16, tag="S_sb")
            for (n0, nw) in N_CHUNKS:
                sps = psS.tile([128, 512], F32, tag="sps")
                for kc in range(KC):
                    nc.tensor.matmul(sps[:mm, :nw],
                                     imgsT_sb[:, kc, r0:r0 + mm],
                                     caps_sb[:, kc, n0:n0 + nw],
                                     start=(kc == 0), stop=False)
                nc.tensor.matmul(sps[:mm, :nw], ones_16[:, :mm],
                                 addsrow_sb[:, n0:n0 + nw],
                                 start=False, stop=True)
                nc.scalar.activation(S_sb[:rt, n0:n0 + nw], sps[:rt, :nw],
                                     AF.Copy)
                nc.vector.tensor_tensor(t[:rt, n0:n0 + nw],
                                        S_sb[:rt, n0:n0 + nw],
                                        bc_scale[:rt, n0:n0 + nw], OP.mult)
            return t, S_sb, invni10, iv_t, ivm1_t

        def v_phase(r0, rt, t, S_sb, invni10):
            """softmax/hard/mixed + num (vector+scalar engines)."""
            t3 = t[:rt, :].rearrange("p (c w) -> p c w", w=W)
            rowmax = small.tile([128, BC], F16, tag="rowmax")
            nc.vector.tensor_reduce(rowmax[:rt, :], t3, axis=AX.X, op=OP.max)
            nrm_all = small.tile([128, 1], F32, tag="nrmall")
            nc.vector.tensor_reduce(nrm_all[:rt, :], rowmax[:rt, :],
                                    axis=AX.X, op=OP.max, negate=True)
            nbias = small.tile([128, 1], F32, tag="nbias")
            nc.vector.tensor_scalar(nbias[:rt, :], nrm_all[:rt, :],
                                    invni10[:rt, :], None, OP.mult)
            el = work.tile([128, CW], F16, tag="el")
            nc.scalar.activation(el[:rt, :], t[:rt, :], AF.Exp,
                                 bias=nbias[:rt, :], scale=invni10[:rt, :])
            el3 = el[:rt, :].rearrange("p (c w) -> p c w", w=W)
            den = small.tile([128, BC], F32, tag="den")
            nc.vector.tensor_reduce(den[:rt, :], el3, axis=AX.X, op=OP.add)
            invden = small.tile([128, BC], F32, tag="invden")
            nc.vector.tensor_scalar(invden[:rt, :], den[:rt, :], oma / am,
                                    oma * TINY / am, OP.mult, OP.add)
            nc.vector.reciprocal(invden[:rt, :], invden[:rt, :])
            soft = work.tile([128, CW], F16, tag="soft")
            s3 = soft[:rt, :].rearrange("p (c w) -> p c w", w=W)
            nc.vector.tensor_tensor(
                s3, el3, invden[:rt, :, None].to_broadcast([rt, BC, W]),
                OP.mult)
            mixed = work.tile([128, CW], F16, tag="mixed")
            m3 = mixed[:rt, :].rearrange("p (c w) -> p c w", w=W)
            # hard into el (dead after soft), then mixed = soft + hard
            nc.vector.tensor_tensor(
                el3, t3, rowmax[:rt, :, None].to_broadcast([rt, BC, W]),
                OP.is_equal)
            nc.vector.tensor_tensor(m3, s3, el3, OP.add)

            # num' = sum_w mixed * S  (prod into soft, dead now)
            nc.vector.tensor_tensor(soft[:rt, :], mixed[:rt, :], S_sb[:rt, :],
                                    OP.mult)
            num = small.tile([128, BC], F32, tag="num")
            nc.vector.tensor_reduce(num[:rt, :], s3, axis=AX.X, op=OP.add)
            return mixed, num

        def qf_pe(r0, rt, mixed):
            """transposes + ups matmuls (PE)."""
            for q in range(NP // 4):
                tps = psT.tile([128, 512], F16, tag="tps")
                for pi in range(4):
                    p = 4 * q + pi
                    c0 = 100 * p
                    tw = min(128, CW - c0)
                    nc.tensor.transpose(tps[0:tw, 128 * pi:128 * pi + rt],
                                        mixed[:rt, c0:c0 + tw],
                                        ident_16[0:rt, 0:rt])
                t4 = tps[:, :].rearrange("p (j x) -> p j x", x=128)
                if q < 7:
                    nc.scalar.activation(
                        M_T[:, 4 * q:4 * q + 4, r0:r0 + rt],
                        t4[:, :, 0:rt], AF.Copy)
                else:
                    nc.scalar.activation(
                        M_T[:, 28:31, r0:r0 + rt],
                        t4[:, 0:3, 0:rt], AF.Copy)
                    nc.scalar.activation(
                        M_T[0:100, 31, r0:r0 + rt],
                        t4[0:100, 3, 0:rt], AF.Copy)
            qprod = work.tile([128, CW], F16, tag="el")  # el ring reuse
            ups_l = []
            for q in range(NP // 4):
                ups = psQ.tile([128, 512], F32, tag="ups")
                for pi in range(4):
                    p = 4 * q + pi
                    nc.tensor.matmul(ups[:rt, 128 * pi:128 * pi + 128],
                                     M_T[:, p, r0:r0 + rt],
                                     Gp[:, p, :], start=True, stop=True)
                ups_l.append(ups)
            return qprod, ups_l

        def qf_fin(r0, rt, mixed, qprod, ups_l, num, iv_t, ivm1_t):
            """qf products (Pool), reduce, out row assembly."""
            for q in range(NP // 4):
                u4 = ups_l[q][:rt, :].rearrange("p (j x) -> p j x", x=128)
                nc.vector.tensor_tensor(
                    qprod[:rt, 400 * q:400 * q + 400].rearrange(
                        "p (j w) -> p j w", w=100),
                    mixed[:rt, 400 * q:400 * q + 400].rearrange(
                        "p (j w) -> p j w", w=100),
                    u4[:, :, 0:100], OP.mult)
            qf = small.tile([128, BC], F32, tag="qf")
            nc.vector.tensor_reduce(
                qf[:rt, :],
                qprod[:rt, :].rearrange("p (c w) -> p c w", w=W),
                axis=AX.X, op=OP.add)

            # out = num/(sqrt(qf) + eps'); invalid rows -> -1
            denom = small.tile([128, BC], F32, tag="denom")
            nc.scalar.activation(denom[:rt, :], qf[:rt, :], AF.Sqrt)
            nc.vector.tensor_scalar(denom[:rt, :], denom[:rt, :], EPS / oma,
                                    None, OP.add)
            nc.vector.reciprocal(denom[:rt, :], denom[:rt, :])
            res = small.tile([128, BC], F32, tag="res")
            nc.vector.tensor_tensor(res[:rt, :], num[:rt, :], denom[:rt, :],
                                    OP.mult)
            nc.vector.tensor_scalar(res[:rt, :], res[:rt, :], iv_t[:rt, :],
                                    ivm1_t[:rt, :], OP.mult, OP.add)

            ops_ = psM.tile([BC, 128], F32, tag="ps")
            nc.tensor.transpose(ops_[:, :rt], res[:rt, :],
                                ident_f32[0:rt, 0:rt])
            nc.scalar.activation(out_sb[:, r0:r0 + rt], ops_[:, :rt], AF.Copy)

        # all S phases up-front (PE stays hot); v/qf_pe/qf_fin staggered
        st = [s_phase(*ROW_TILES[i]) for i in range(3)]
        pend = None
        for i in range(3):
            r0, rt = ROW_TILES[i]
            mi, ni = v_phase(r0, rt, *st[i][:3])
            if pend is not None:
                qf_fin(*pend)
            qp, ul = qf_pe(r0, rt, mi)
            pend = (r0, rt, mi, qp, ul, ni, st[i][3], st[i][4])
        qf_fin(*pend)

        # single final output DMA: out_sb [BC, ROWS] -> out_ext [BI, BC, R]
        nc.scalar.dma_start(
            out=out_ext.rearrange("i c r -> c i r"),
            in_=out_sb[:].rearrange("c (i r) -> c i r", r=R))

    nc.finalize()
    return nc


def _get_runner(a: float):
    key = round(float(a), 9)
    if key not in _CACHE:
        _CACHE[key] = _build(key)
    return _CACHE[key]


def _gcaps_padded(capsT, core):
    """[D, PPC*128]: pair j -> even cap words at cols 128j+0:50, odd cap
    words at 128j+64:114, rest zero (32-aligned PSUM block reads)."""
    g = np.zeros((D, GW), dtype=np.float16)
    base = core * CPC * W
    for j in range(PPC):
        g[:, 128 * j:128 * j + 50] = capsT[:, base + 100 * j:
                                           base + 100 * j + 50]
        g[:, 128 * j + 64:128 * j + 114] = capsT[:, base + 100 * j + 50:
                                                 base + 100 * j + 100]
    return g


def _host_prep(imgs, caps, img_lens, cap_lens):
    imgs = np.ascontiguousarray(np.asarray(imgs, dtype=np.float32))
    caps = np.ascontiguousarray(np.asarray(caps, dtype=np.float32))
    img_lens = np.asarray(img_lens).astype(np.int64)
    cap_lens = np.asarray(cap_lens).astype(np.int64)

    capsT = np.ascontiguousarray(
        caps.reshape(BC * W, D).T).astype(np.float16)   # [D, CW]
    cap_mask = (np.arange(W)[:, None] < cap_lens[None, :]).astype(np.float32)
    cm_cw = cap_mask.T.reshape(1, CW)
    adds_row = np.where(cm_cw > 0, 0.0, NEGS).astype(np.float16)
    inv_nc = 1.0 / (np.linalg.norm(caps.astype(np.float64), axis=-1) + EPS)
    scale_row = (inv_nc.reshape(1, CW) * cm_cw
                 + KMASK * (1.0 - cm_cw)).astype(np.float16)
    bc_scale_in = np.ascontiguousarray(
        np.broadcast_to(scale_row, (128, CW)))

    in_maps = []
    for core in range(N_CORES):
        sl = slice(core * BI, (core + 1) * BI)
        im = imgs[sl].reshape(ROWS, D)
        imT = np.ascontiguousarray(im.T).astype(np.float16)
        iv = (np.arange(R)[None, :] < img_lens[sl][:, None]).astype(
            np.float32).reshape(ROWS, 1)
        in_maps.append({
            "capsT": capsT,
            "gcaps": _gcaps_padded(capsT, core),
            "imgsT": imT,
            "imgs_nat": im,
            "bc_scale_in": bc_scale_in,
            "adds_row": adds_row,
            "iv_col": iv,
            "ivm1_col": iv - 1.0,
        })
    return in_maps


def run_on_device(inputs: dict, trace: bool = False):
    """Returns (output [64,64,36] f32, BassKernelResults)."""
    from concourse.bass_utils import run_bass_kernel_spmd
    alpha = float(np.asarray(inputs["alpha"]).reshape(-1)[0])
    a = 1.0 / (1.0 + np.exp(-alpha))
    nc = _get_runner(a)
    in_maps = _host_prep(inputs["imgs"], inputs["caps"], inputs["img_lens"],
                         inputs["cap_lens"])
    r = run_bass_kernel_spmd(nc, in_maps, list(range(N_CORES)), trace=trace)
    out = np.concatenate([r.results[c]["out"][None] for c in range(N_CORES)],
                         axis=0)
    return out.reshape(B, BC, R).astype(np.float32), r


def kernel(imgs, caps, img_lens, cap_lens, alpha):
    out, _ = run_on_device({"imgs": imgs, "caps": caps, "img_lens": img_lens,
                            "cap_lens": cap_lens, "alpha": alpha})
    return out


# revision 11
# speedup vs baseline: 1.0064x; 1.0064x over previous
"""Trainium2 Bass kernel for nn_AdaptiveMixedCoding (8 NeuronCores).

Sharding: data-parallel over B_img (8 images per core); caps/cap_lens/alpha
replicated. Caption Grams computed split across cores and AllGathered.

Caption length-grouping: the NA=32 shortest captions (len <= WA=32) are
stored in 32-wide word slots ("A", 4 caps per 128-col unit at 32-offsets);
the rest keep 50-wide slots ("B", 2 caps per unit, packed 100 in the
similarity matrix). This shrinks the working width CWP from 3200 to
NA*32 + NB*50 = 2624, cutting S-matmul streaming, every elementwise/reduce
pass, and the qf unit count proportionally.

Per-core algorithm (Bi=8 imgs, R=36 regions, Bc=64 caps):
  S[row, cw] = dot(imgs[row], caps'[cw]) + adds  (fp16 matmul, K=1 ones row
               adds 0 valid / -30000 masked/pad into the same PSUM group)
  t          = S_sb * bc_scale'   (fp16; masked t ~= -300)
  exp        = Exp(t*invni10 - rowmax_all*invni10)  per-ROW max as scalar
               bias (per-caption max cancels in the softmax)
  hard       = (t == rowmax_c)  per-caption fp16 max, exact compare
  mixed'     = hard + exp * a/((1-a) den)
  num'       = sum_w mixed' * S_sb
  qf'        = mixed'^T G mixed'  (per-unit transposes -> M_T, u = M_T^T Gp,
               4 units per PSUM bank, strided products, 2 reduces)
  out        = num'/(sqrt(qf') + eps/(1-a)), invalid img rows -> -1

End-to-end l2 rel err vs the f32 reference ~8e-3 (fp16 t argmax ties).
"""
import sys
import contextlib

sys.path.insert(0, '/opt/trn_rl_repo')

import numpy as np

from concourse import bacc, tile, mybir

F32 = mybir.dt.float32
F16 = mybir.dt.float16
AF = mybir.ActivationFunctionType
OP = mybir.AluOpType
AX = mybir.AxisListType

N_CORES = 8
B, R, W, D = 64, 36, 50, 1024
BC = B
BI = B // N_CORES
ROWS = BI * R               # 288
KC = D // 128               # 8 contraction chunks
WA = 32                     # A-group stored words per caption
ROW_TILES = [(0, 128), (128, 128), (256, 32)]
EPS = 1e-8
NEGS = -30000.0             # masked S offset; fp16-safe, *0.01 -> t ~= -300
KMASK = 0.01
TINY = 1e-30

_CACHE = {}


def _params(na):
    nb = BC - na
    cwa = na * WA            # A region width
    cwp = cwa + nb * W       # total packed width
    nu = na // 4 + nb // 2   # 128-col units (A: 4 caps, B: 2 caps)
    upc = nu // N_CORES      # gram units per core
    chunks = [(i * 512, min(512, cwp - i * 512))
              for i in range((cwp + 511) // 512)]
    return nb, cwa, cwp, nu, upc, chunks


def _build(a, na):
    am = max(a, 1e-6)
    oma = max(1.0 - a, 1e-6)
    NB, CWA, CWP, NU, UPC, N_CHUNKS = _params(na)
    NUA = na // 4            # A units
    GW = UPC * 128

    nc = bacc.Bacc("TRN2", target_bir_lowering=False, debug=False,
                   num_devices=N_CORES)

    capsT = nc.declare_dram_parameter("capsT", [D, CWP], F16, isOutput=False)
    gcaps = nc.declare_dram_parameter("gcaps", [D, GW], F16, isOutput=False)
    gmask = nc.declare_dram_parameter("gmask", [128, GW], F16, isOutput=False)
    imgsT = nc.declare_dram_parameter("imgsT", [D, ROWS], F16, isOutput=False)
    imgs_nat = nc.declare_dram_parameter("imgs_nat", [ROWS, D], F32,
                                         isOutput=False)
    bc_scale_in = nc.declare_dram_parameter("bc_scale_in", [128, CWP], F16,
                                            isOutput=False)  # invnc / KMASK
    adds_row = nc.declare_dram_parameter("adds_row", [1, CWP], F16,
                                         isOutput=False)    # 0 / NEGS
    iv_col = nc.declare_dram_parameter("iv_col", [ROWS, 1], F32,
                                       isOutput=False)
    ivm1_col = nc.declare_dram_parameter("ivm1_col", [ROWS, 1], F32,
                                         isOutput=False)
    out_ext = nc.declare_dram_parameter("out", [BI, BC, R], F32, isOutput=True)

    gb_in = nc.dram_tensor("gb_in", [UPC, 128, 128], F16)
    gb_out = nc.dram_tensor("gb_out", [NU, 128, 128], F16,
                            addr_space="Shared")
    gb2 = nc.dram_tensor("gb2", [NU - NUA, 128, 128], F16)  # B row-remap

    with tile.TileContext(nc) as tc, contextlib.ExitStack() as ctx:
        const = ctx.enter_context(tc.tile_pool(name="const", bufs=1))
        big = ctx.enter_context(tc.tile_pool(name="big", bufs=1))
        work = ctx.enter_context(tc.tile_pool(name="work", bufs=2))
        work3 = ctx.enter_context(tc.tile_pool(name="work3", bufs=3))
        scr = ctx.enter_context(tc.tile_pool(name="scr", bufs=1))
        small = ctx.enter_context(tc.tile_pool(name="small", bufs=2))
        psS = ctx.enter_context(tc.tile_pool(name="psS", bufs=2, space="PSUM"))
        psQ = ctx.enter_context(tc.tile_pool(name="psQ", bufs=2, space="PSUM"))
        psM = ctx.enter_context(tc.tile_pool(name="psM", bufs=2, space="PSUM"))
        psT = ctx.enter_context(tc.tile_pool(name="psT", bufs=2, space="PSUM"))

        # ---- constants --------------------------------------------------
        ident_16 = const.tile([128, 128], F16)
        from concourse.masks import make_identity
        make_identity(nc, ident_16[:])
        ident_f32 = const.tile([128, 128], F32)
        make_identity(nc, ident_f32[:])
        ones_16 = const.tile([1, 128], F16)
        nc.gpsimd.memset(ones_16[:], 1.0)

        # ---- input loads (priority order) -------------------------------
        gcaps_sb = big.tile([128, KC, GW], F16)
        gcaps_r = gcaps.rearrange("(k p) m -> p k m", p=128)
        for j in range(UPC):
            nc.sync.dma_start(out=gcaps_sb[:, :, 128 * j:128 * j + 128],
                              in_=gcaps_r[:, :, 128 * j:128 * j + 128])
        imgsT_sb = big.tile([128, KC, ROWS], F16)
        nc.sync.dma_start(out=imgsT_sb[:],
                          in_=imgsT.rearrange("(k p) m -> p k m", p=128))
        caps_sb = big.tile([128, KC, CWP], F16)
        capsT_r = capsT.rearrange("(k p) m -> p k m", p=128)
        for (n0, nw) in N_CHUNKS:
            nc.sync.dma_start(out=caps_sb[:, :, n0:n0 + nw],
                              in_=capsT_r[:, :, n0:n0 + nw])
        bc_scale = big.tile([128, CWP], F16)
        nc.sync.dma_start(out=bc_scale[:], in_=bc_scale_in[:])

        addsrow_sb = const.tile([1, CWP], F16)
        nc.gpsimd.dma_start(out=addsrow_sb[:], in_=adds_row[:])
        gmask_sb = const.tile([128, GW], F16)
        nc.gpsimd.dma_start(out=gmask_sb[:], in_=gmask[:])

        # ---- Grams for this core's UPC units ----------------------------
        # Gloc[:, j, :] = (gcaps_j^T gcaps_j) * gmask_j  (cross blocks -> 0)
        Gloc = big.tile([128, UPC, 128], F16)
        for j in range(UPC):
            c0 = j * 128
            gps = psM.tile([128, 128], F32, tag="ps")
            for kc in range(KC):
                nc.tensor.matmul(gps[:, :],
                                 gcaps_sb[:, kc, c0:c0 + 128],
                                 gcaps_sb[:, kc, c0:c0 + 128],
                                 start=(kc == 0), stop=(kc == KC - 1))
            nc.vector.tensor_tensor(Gloc[:, j, :], gps[:, :],
                                    gmask_sb[:, c0:c0 + 128], OP.mult)

        # gather all units (overlaps the S matmuls; needed only by qf)
        nc.gpsimd.dma_start(
            out=gb_in.rearrange("j r b -> r j b"),
            in_=Gloc[:])
        nc.gpsimd.collective_compute(
            "AllGather", OP.bypass,
            replica_groups=[list(range(N_CORES))],
            ins=[gb_in[:].opt()],
            outs=[gb_out[:].opt()],
        )
        # B-unit gram rows live at 64:114 (32-aligned compute layout) but
        # M_T packs the odd caption at rows 50:100 -> remap via DRAM copies.
        zb = const.tile([128, (NU - NUA) * 128], F16)
        nc.vector.memset(zb[:], 0.0)
        nc.gpsimd.dma_start(
            out=gb2.rearrange("j r b -> (j r b)")[None, :],
            in_=zb[:])
        nc.gpsimd.dma_start(out=gb2[:, 0:50, :], in_=gb_out[NUA:, 0:50, :])
        nc.gpsimd.dma_start(out=gb2[:, 50:100, :], in_=gb_out[NUA:, 64:114, :])
        Gp = big.tile([128, NU, 128], F16)
        if NUA:
            nc.sync.dma_start(
                out=Gp[:, 0:NUA, :],
                in_=gb_out[0:NUA, :, :].rearrange("j r b -> r j b"))
        nc.sync.dma_start(
            out=Gp[:, NUA:, :],
            in_=gb2[:, :, :].rearrange("j r b -> r j b"))

        # transposed mixed, unit-block layout (built per row tile)
        M_T = big.tile([128, NU, ROWS], F16)
        nc.vector.memset(M_T[:, NU - 1, :], 0.0)

        # persistent output accumulator [BC, ROWS]
        out_sb = big.tile([BC, ROWS], F32)

        # unit table: (mixed col start, transpose width)
        units = [(128 * u, 128) for u in range(NUA)]
        units += [(CWA + 100 * v, min(128, CWP - CWA - 100 * v))
                  for v in range(NU - NUA)]

        # ---- pipelined per-row-tile phases ------------------------------
        def s_phase(r0, rt):
            mm = 128 if (r0 + 128 <= ROWS) else rt
            img_nat_t = work.tile([128, D], F32, tag="imgnat")
            nc.sync.dma_start(out=img_nat_t[:rt, :],
                              in_=imgs_nat[r0:r0 + rt, :])
            sq_scr = scr.tile([128, D], F32, tag="sqscr")
            nsq_img = small.tile([128, 1], F32, tag="nsqimg")
            nc.scalar.activation(sq_scr[:rt, :], img_nat_t[:rt, :], AF.Square,
                                 accum_out=nsq_img[:rt, :])
            invni10 = small.tile([128, 1], F32, tag="invni10")
            nc.scalar.activation(invni10[:rt, :], nsq_img[:rt, :], AF.Sqrt,
                                 scale=0.01)
            nc.vector.reciprocal(invni10[:rt, :], invni10[:rt, :])
            iv_t = small.tile([128, 1], F32, tag="ivt")
            nc.gpsimd.dma_start(out=iv_t[:rt, :], in_=iv_col[r0:r0 + rt, :])
            ivm1_t = small.tile([128, 1], F32, tag="ivm1t")
            nc.gpsimd.dma_start(out=ivm1_t[:rt, :],
                                in_=ivm1_col[r0:r0 + rt, :])

            t = work3.tile([128, CWP], F16, tag="t")
            S_sb = work3.tile([128, CWP], F16, tag="S_sb")
            for (n0, nw) in N_CHUNKS:
                sps = psS.tile([128, 512], F32, tag="sps")
                for kc in range(KC):
                    nc.tensor.matmul(sps[:mm, :nw],
                                     imgsT_sb[:, kc, r0:r0 + mm],
                                     caps_sb[:, kc, n0:n0 + nw],
                                     start=(kc == 0), stop=False)
                nc.tensor.matmul(sps[:mm, :nw], ones_16[:, :mm],
                                 addsrow_sb[:, n0:n0 + nw],
                                 start=False, stop=True)
                nc.scalar.activation(S_sb[:rt, n0:n0 + nw], sps[:rt, :nw],
                                     AF.Copy)
                nc.vector.tensor_tensor(t[:rt, n0:n0 + nw],
                                        S_sb[:rt, n0:n0 + nw],
                                        bc_scale[:rt, n0:n0 + nw], OP.mult)
            return t, S_sb, invni10, iv_t, ivm1_t

        def _ab(x, rt):
            """split [rt, CWP] into per-caption 3d views (A, B)."""
            xa = (x[:rt, 0:CWA].rearrange("p (c w) -> p c w", w=WA)
                  if na else None)
            xb = x[:rt, CWA:CWP].rearrange("p (c w) -> p c w", w=W)
            return xa, xb

        def v_phase(r0, rt, t, S_sb, invni10):
            ta, tb = _ab(t, rt)
            rowmax = small.tile([128, BC], F16, tag="rowmax")
            if na:
                nc.vector.tensor_reduce(rowmax[:rt, 0:na], ta,
                                        axis=AX.X, op=OP.max)
            nc.vector.tensor_reduce(rowmax[:rt, na:BC], tb,
                                    axis=AX.X, op=OP.max)
            nrm_all = small.tile([128, 1], F32, tag="nrmall")
            nc.vector.tensor_reduce(nrm_all[:rt, :], rowmax[:rt, :],
                                    axis=AX.X, op=OP.max, negate=True)
            nbias = small.tile([128, 1], F32, tag="nbias")
            nc.vector.tensor_scalar(nbias[:rt, :], nrm_all[:rt, :],
                                    invni10[:rt, :], None, OP.mult)
            el = work.tile([128, CWP], F16, tag="el")
            nc.scalar.activation(el[:rt, :], t[:rt, :], AF.Exp,
                                 bias=nbias[:rt, :], scale=invni10[:rt, :])
            ela, elb = _ab(el, rt)
            den = small.tile([128, BC], F32, tag="den")
            if na:
                nc.vector.tensor_reduce(den[:rt, 0:na], ela,
                                        axis=AX.X, op=OP.add)
            nc.vector.tensor_reduce(den[:rt, na:BC], elb,
                                    axis=AX.X, op=OP.add)
            invden = small.tile([128, BC], F32, tag="invden")
            nc.vector.tensor_scalar(invden[:rt, :], den[:rt, :], oma / am,
                                    oma * TINY / am, OP.mult, OP.add)
            nc.vector.reciprocal(invden[:rt, :], invden[:rt, :])
            soft = work.tile([128, CWP], F16, tag="soft")
            sa, sb = _ab(soft, rt)
            if na:
                nc.vector.tensor_tensor(
                    sa, ela,
                    invden[:rt, 0:na, None].to_broadcast([rt, na, WA]),
                    OP.mult)
            nc.vector.tensor_tensor(
                sb, elb,
                invden[:rt, na:BC, None].to_broadcast([rt, NB, W]),
                OP.mult)
            mixed = work.tile([128, CWP], F16, tag="mixed")
            # hard into el (dead after soft), then mixed = soft + hard
            if na:
                nc.vector.tensor_tensor(
                    ela, ta,
                    rowmax[:rt, 0:na, None].to_broadcast([rt, na, WA]),
                    OP.is_equal)
            nc.vector.tensor_tensor(
                elb, tb,
                rowmax[:rt, na:BC, None].to_broadcast([rt, NB, W]),
                OP.is_equal)
            nc.vector.tensor_tensor(mixed[:rt, :], soft[:rt, :], el[:rt, :],
                                    OP.add)

            # num' = sum_w mixed * S  (prod into soft, dead now)
            nc.vector.tensor_tensor(soft[:rt, :], mixed[:rt, :], S_sb[:rt, :],
                                    OP.mult)
            pa, pb = _ab(soft, rt)
            num = small.tile([128, BC], F32, tag="num")
            if na:
                nc.vector.tensor_reduce(num[:rt, 0:na], pa,
                                        axis=AX.X, op=OP.add)
            nc.vector.tensor_reduce(num[:rt, na:BC], pb,
                                    axis=AX.X, op=OP.add)
            return mixed, num

        def qf_pe(r0, rt, mixed):
            """transposes + ups matmuls (PE), 4 units per PSUM bank."""
            for q in range(NU // 4):
                tps = psT.tile([128, 512], F16, tag="tps")
                tws = []
                for pi in range(4):
                    u = 4 * q + pi
                    c0, tw = units[u]
                    tws.append(tw)
                    nc.tensor.transpose(tps[0:tw, 128 * pi:128 * pi + rt],
                                        mixed[:rt, c0:c0 + tw],
                                        ident_16[0:rt, 0:rt])
                t4 = tps[:, :].rearrange("p (j x) -> p j x", x=128)
                if min(tws) == 128:
                    nc.scalar.activation(
                        M_T[:, 4 * q:4 * q + 4, r0:r0 + rt],
                        t4[:, :, 0:rt], AF.Copy)
                else:
                    nc.scalar.activation(
                        M_T[:, 4 * q:4 * q + 3, r0:r0 + rt],
                        t4[:, 0:3, 0:rt], AF.Copy)
                    nc.scalar.activation(
                        M_T[0:tws[3], 4 * q + 3, r0:r0 + rt],
                        t4[0:tws[3], 3, 0:rt], AF.Copy)
            ups_l = []
            for q in range(NU // 4):
                ups = psQ.tile([128, 512], F32, tag="ups")
                for pi in range(4):
                    u = 4 * q + pi
                    nc.tensor.matmul(ups[:rt, 128 * pi:128 * pi + 128],
                                     M_T[:, u, r0:r0 + rt],
                                     Gp[:, u, :], start=True, stop=True)
                ups_l.append(ups)
            return ups_l

        def qf_fin(r0, rt, mixed, ups_l, num, iv_t, ivm1_t):
            """qf products, reduces, out row assembly."""
            qprod = work.tile([128, CWP], F16, tag="el")  # el ring reuse
            for q in range(NU // 4):
                if 4 * q < NUA:
                    # A group: fully packed 512 cols
                    cq = 512 * q
                    nc.vector.tensor_tensor(
                        qprod[:rt, cq:cq + 512],
                        mixed[:rt, cq:cq + 512],
                        ups_l[q][:rt, 0:512], OP.mult)
                else:
                    # B group: caps at 64-offsets in ups, packed 50 in mixed
                    cq = CWA + 400 * (q - NUA // 4)
                    u8 = ups_l[q][:rt, :].rearrange(
                        "p (j c) -> p j c", c=64)
                    nc.vector.tensor_tensor(
                        qprod[:rt, cq:cq + 400].rearrange(
                            "p (j w) -> p j w", w=W),
                        mixed[:rt, cq:cq + 400].rearrange(
                            "p (j w) -> p j w", w=W),
                        u8[:, :, 0:W], OP.mult)
            qa, qb = _ab(qprod, rt)
            qf = small.tile([128, BC], F32, tag="qf")
            if na:
                nc.vector.tensor_reduce(qf[:rt, 0:na], qa,
                                        axis=AX.X, op=OP.add)
            nc.vector.tensor_reduce(qf[:rt, na:BC], qb,
                                    axis=AX.X, op=OP.add)

            denom = small.tile([128, BC], F32, tag="denom")
            nc.scalar.activation(denom[:rt, :], qf[:rt, :], AF.Sqrt)
            nc.vector.tensor_scalar(denom[:rt, :], denom[:rt, :], EPS / oma,
                                    None, OP.add)
            nc.vector.reciprocal(denom[:rt, :], denom[:rt, :])
            res = small.tile([128, BC], F32, tag="res")
            nc.vector.tensor_tensor(res[:rt, :], num[:rt, :], denom[:rt, :],
                                    OP.mult)
            nc.vector.tensor_scalar(res[:rt, :], res[:rt, :], iv_t[:rt, :],
                                    ivm1_t[:rt, :], OP.mult, OP.add)

            ops_ = psM.tile([BC, 128], F32, tag="ps")
            nc.tensor.transpose(ops_[:, :rt], res[:rt, :],
                                ident_f32[0:rt, 0:rt])
            nc.scalar.activation(out_sb[:, r0:r0 + rt], ops_[:, :rt], AF.Copy)

        # all S phases up-front (PE stays hot); v/qf_pe/qf_fin staggered
        st = [s_phase(*ROW_TILES[i]) for i in range(3)]
        pend = None
        for i in range(3):
            r0, rt = ROW_TILES[i]
            mi, ni = v_phase(r0, rt, *st[i][:3])
            if pend is not None:
                qf_fin(*pend)
            ul = qf_pe(r0, rt, mi)
            pend = (r0, rt, mi, ul, ni, st[i][3], st[i][4])
        qf_fin(*pend)

        # single final output DMA (captions in permuted order; host unmaps)
        nc.scalar.dma_start(
            out=out_ext.rearrange("i c r -> c i r"),
            in_=out_sb[:].rearrange("c (i r) -> c i r", r=R))

    nc.finalize()
    return nc


def _get_runner(a, na):
    key = (round(float(a), 9), na)
    if key not in _CACHE:
        _CACHE[key] = _build(*key)
    return _CACHE[key]


def _host_prep(imgs, caps, img_lens, cap_lens, na, perm):
    NB, CWA, CWP, NU, UPC, _ = _params(na)
    NUA = na // 4
    imgs = np.ascontiguousarray(np.asarray(imgs, dtype=np.float32))
    caps = np.ascontiguousarray(np.asarray(caps, dtype=np.float32))
    img_lens = np.asarray(img_lens).astype(np.int64)
    cap_lens = np.asarray(cap_lens).astype(np.int64)

    capsTf = np.ascontiguousarray(
        caps.reshape(BC * W, D).T).reshape(D, BC, W)  # [D, c, w] f32
    # packed layout: A caps (perm[:na]) in 32-wide slots, B in 50-wide
    capsT = np.zeros((D, CWP), dtype=np.float16)
    valid = np.zeros(CWP, dtype=np.float32)
    inv_nc = 1.0 / (np.linalg.norm(caps.astype(np.float64), axis=-1) + EPS)
    scale = np.full(CWP, KMASK, dtype=np.float32)
    for j, c in enumerate(perm):
        if j < na:
            sl = slice(WA * j, WA * j + WA)
            ww = WA
        else:
            sl = slice(CWA + W * (j - na), CWA + W * (j - na) + W)
            ww = W
        capsT[:, sl] = capsTf[:, c, 0:ww].astype(np.float16)
        v = (np.arange(ww) < cap_lens[c]).astype(np.float32)
        valid[sl] = v
        scale[sl] = np.where(v > 0, inv_nc[c, 0:ww], KMASK)

    adds_row = np.where(valid > 0, 0.0, NEGS).astype(np.float16)[None, :]
    bc_scale_in = np.ascontiguousarray(
        np.broadcast_to(scale.astype(np.float16)[None, :], (128, CWP)))

    # gram inputs per core: UPC units of 128 cols
    # A unit u: caps perm[4u..4u+4] at 32-offsets == capsT slice
    # B unit v: caps perm[na+2v], perm[na+2v+1] at 0:50 / 64:114
    gmask_u = np.zeros((NU, 128, 128), dtype=np.float16)
    gcaps_u = np.zeros((NU, D, 128), dtype=np.float16)
    for u in range(NUA):
        gcaps_u[u] = capsT[:, 128 * u:128 * u + 128]
        for aa in range(4):
            gmask_u[u, 32 * aa:32 * aa + 32, 32 * aa:32 * aa + 32] = 1.0
    for v in range(NU - NUA):
        u = NUA + v
        c0, c1 = perm[na + 2 * v], perm[na + 2 * v + 1]
        gcaps_u[u, :, 0:50] = capsTf[:, c0, :].astype(np.float16)
        gcaps_u[u, :, 64:114] = capsTf[:, c1, :].astype(np.float16)
        gmask_u[u, 0:50, 0:50] = 1.0
        gmask_u[u, 64:114, 64:114] = 1.0

    in_maps = []
    for core in range(N_CORES):
        sl = slice(core * BI, (core + 1) * BI)
        im = imgs[sl].reshape(ROWS, D)
        imT = np.ascontiguousarray(im.T).astype(np.float16)
        iv = (np.arange(R)[None, :] < img_lens[sl][:, None]).astype(
            np.float32).reshape(ROWS, 1)
        usl = slice(core * UPC, (core + 1) * UPC)
        in_maps.append({
            "capsT": capsT,
            "gcaps": np.ascontiguousarray(
                gcaps_u[usl].transpose(1, 0, 2).reshape(D, UPC * 128)),
            "gmask": np.ascontiguousarray(
                gmask_u[usl].transpose(1, 0, 2).reshape(128, UPC * 128)),
            "imgsT": imT,
            "imgs_nat": im,
            "bc_scale_in": bc_scale_in,
            "adds_row": adds_row,
            "iv_col": iv,
            "ivm1_col": iv - 1.0,
        })
    return in_maps


def run_on_device(inputs: dict, trace: bool = False):
    """Returns (output [64,64,36] f32, BassKernelResults)."""
    from concourse.bass_utils import run_bass_kernel_spmd
    alpha = float(np.asarray(inputs["alpha"]).reshape(-1)[0])
    a = 1.0 / (1.0 + np.exp(-alpha))
    cap_lens = np.asarray(inputs["cap_lens"]).astype(np.int64)
    order = np.argsort(cap_lens, kind="stable")
    na = 32 if cap_lens[order[31]] <= WA else 0
    perm = np.asarray(order if na else np.arange(BC))
    nc = _get_runner(a, na)
    in_maps = _host_prep(inputs["imgs"], inputs["caps"], inputs["img_lens"],
                         cap_lens, na, perm)
    r = run_bass_kernel_spmd(nc, in_maps, list(range(N_CORES)), trace=trace)
    dev = np.concatenate([r.results[c]["out"][None] for c in range(N_CORES)],
                         axis=0).reshape(B, BC, R).astype(np.float32)
    out = np.empty_like(dev)
    out[:, perm, :] = dev
    return out, r


def kernel(imgs, caps, img_lens, cap_lens, alpha):
    out, _ = run_on_device({"imgs": imgs, "caps": caps, "img_lens": img_lens,
                            "cap_lens": cap_lens, "alpha": alpha})
    return out
